# revision 1
# baseline (speedup 1.0000x reference)
"""Trainium2 Bass kernel: multi-head attention (B=2, S=2048, D=1024, H=16,
d_k=64) with RoPE and masked softmax, sharded over 8 NeuronCores as
(batch x head-group): core = b*4 + g handles batch b, heads [4g, 4g+4).

Per-core device program (all matmuls float32r = full-rate fp32 on the PE):
  1. Projections: QT/KT in [feature, seq] layout (x, W host-pretransposed;
     1/sqrt(d_k) folded into W_q); V directly in [seq, feature] via
     column-slab x loads. Input DMAs split across both HWDGE queues.
  2. RoPE on QT/KT: rotate-half via 32-row SBUF->SBUF DMAs, then
     mul/mul/add on VectorE against host-built cos / sign-folded sin.
  3. A ones column per 65-wide V head block makes the PV matmul emit
     softmax denominators for free (M=65).
  4. scores^T = K_h(dk x S) x Q_h -> [k, q] PSUM tiles (no softmax max
     subtraction: |scores| = O(10)); causal mask pre-exp on the f32 PSUM
     (DVE -1e9 memset + host triangular tile add); exp on ScalarE;
     ctx^T accumulates over k-tiles at PSUM-bank granularity with causal
     bank skipping. The kt loop is software-pipelined (PV lags one step)
     so the in-order PE stream is never parked behind exp.
  5. 1/sumexp broadcast across partitions via a K=1 PE outer product;
     normalized ctx^T feeds the W_o shard matmul -> partial (S, D) output.
Host sums the 4 group partials per batch. Wait counts >1 are hoisted onto
single-wait no-ops after scheduling (walrus codegen limitation).
"""
import sys

sys.path.insert(0, "/opt/trn_rl_repo")

from contextlib import ExitStack

import numpy as np

import concourse.bass as bass
import concourse.mybir as mybir
import concourse.tile as tile
FP = mybir.dt.float32
FPR = mybir.dt.float32r
EXP = mybir.ActivationFunctionType.Exp

D = 1024        # d_model
S = 2048        # sequence length
NB = 2          # batches
HPG = 4         # heads per group (= per core)
DK = 64         # head dim
F = HPG * DK    # 256 = group feature width
KT = D // 128   # 8 contraction tiles for projections
ST = S // 128   # 16 seq tiles
QCW = 1024      # q-chunk width (= 2 PSUM banks)
NQC = S // QCW  # 2
NEG = -1e9

_nc_cache = {}


def _mm(nc, out, lhsT, rhs, **kw):
    nc.tensor.matmul(out, lhsT, rhs, **kw)


def _hoist_waits(nc):
    """Several walrus codegen structs (fused-LDW fp32/fp32r matmul, pseudo
    direct2d DMA, ...) only have room for a single sync wait. Hoist every
    limited instruction's waits (when >1) onto same-engine no-ops inserted
    just before it."""
    f = nc.m.functions[0]

    def engine_builder(eng):
        return {
            mybir.EngineType.PE: nc.tensor,
            mybir.EngineType.DVE: nc.vector,
            mybir.EngineType.Activation: nc.scalar,
            mybir.EngineType.Pool: nc.gpsimd,
            mybir.EngineType.SP: nc.sync,
        }[eng]

    def fresh_nop(eng):
        inst = engine_builder(eng).nop().ins
        for b in f.blocks:
            for i, x in enumerate(b.instructions):
                if x is inst:
                    del b.instructions[i]
                    return inst
        raise RuntimeError("created nop not found in any block")

    total = 0
    for blk in f.blocks:
        out = []
        for inst in blk.instructions:
            si = inst.sync_info
            if si is not None and len(si.on_wait) > 1:
                for w in si.on_wait[:-1]:
                    nop = fresh_nop(inst.engine)
                    nop.sync_info = mybir.SyncInfo(on_wait=[w], on_update=[])
                    out.append(nop)
                    total += 1
                inst.sync_info = mybir.SyncInfo(on_wait=[si.on_wait[-1]],
                                                on_update=list(si.on_update))
            out.append(inst)
        blk.instructions[:] = out
    return total


def build_nc(mask_mode):
    """mask_mode: 'causal' | 'full' | 'general'."""
    assert mask_mode in ("causal", "full", "general")
    nc = bass.Bass("TRN2", target_bir_lowering=False, debug=False, num_devices=8)

    xqT = nc.dram_tensor("xqT", [D, S], FPR, kind="ExternalInput").ap()
    xkT = nc.dram_tensor("xkT", [D, S], FPR, kind="ExternalInput").ap()
    xvT = nc.dram_tensor("xvT", [D, S], FPR, kind="ExternalInput").ap()
    wqT = nc.dram_tensor("wqT", [D, F], FPR, kind="ExternalInput").ap()
    wkT = nc.dram_tensor("wkT", [D, F], FPR, kind="ExternalInput").ap()
    wvT = nc.dram_tensor("wvT", [D, F], FPR, kind="ExternalInput").ap()
    woT = nc.dram_tensor("woT", [F, D], FPR, kind="ExternalInput").ap()
    cosd = nc.dram_tensor("cosS", [128, S], FP, kind="ExternalInput").ap()
    sind = nc.dram_tensor("sinS", [128, S], FP, kind="ExternalInput").ap()
    if mask_mode == "general":
        biasT = nc.dram_tensor("biasT", [S, S], FP, kind="ExternalInput").ap()
    if mask_mode == "causal":
        triD = nc.dram_tensor("triD", [128, 128], FP, kind="ExternalInput").ap()
    outp = nc.dram_tensor("outp", [S, D], FP, kind="ExternalOutput").ap()

    with tile.TileContext(nc) as tc, ExitStack() as ctx:
        const = ctx.enter_context(tc.tile_pool(name="const", bufs=1))
        qk = ctx.enter_context(tc.tile_pool(name="qk", bufs=1))

        wq_sb = const.tile([128, KT * F], FPR)
        wk_sb = const.tile([128, KT * F], FPR)
        wv_sb = const.tile([128, KT * F], FPR)
        wo_sb = const.tile([128, 2 * D], FPR)
        cos_sb = const.tile([128, S], FP)
        sin_sb = const.tile([128, S], FP)
        ones_f32 = const.tile([1, 64], FP)
        nc.vector.memset(ones_f32[:], 1.0)
        ones_col = const.tile([1, 64], FPR)
        nc.scalar.copy(ones_col[:], ones_f32[:])
        ones64 = const.tile([128, 64], FP)
        nc.vector.memset(ones64[:], 1.0)
        # single-DMA weight loads (multiple DMAs into one tile would attach
        # too many sem waits to the first fused-LDW matmul for walrus)
        nc.scalar.dma_start(wq_sb[:].rearrange("p (k f) -> p k f", k=KT),
                            wqT[:].rearrange("(k p) f -> p k f", p=128))
        nc.scalar.dma_start(wk_sb[:].rearrange("p (k f) -> p k f", k=KT),
                            wkT[:].rearrange("(k p) f -> p k f", p=128))
        nc.scalar.dma_start(wv_sb[:].rearrange("p (k f) -> p k f", k=KT),
                            wvT[:].rearrange("(k p) f -> p k f", p=128))
        nc.scalar.dma_start(wo_sb[:].rearrange("p (t e) -> p t e", t=2),
                            woT[:].rearrange("(t p) e -> p t e", p=128))
        nc.scalar.dma_start(cos_sb[:], cosd[:])
        nc.scalar.dma_start(sin_sb[:], sind[:])
        if mask_mode == "causal":
            tri_sb = const.tile([128, 128], FP)
            nc.scalar.dma_start(tri_sb[:], triD[:])

        # persistent activations: [p, t*S + s] layouts (t-tile 0: heads 0,1;
        # t-tile 1: heads 2,3 of the group)
        qt_sb = qk.tile([128, 2 * S], FPR)
        kt_sb = qk.tile([128, 2 * S], FPR)
        # V in [s, f] layout with a ones column per head: 65-wide head blocks
        v_sb = qk.tile([128, ST * HPG * 65], FPR)
        ctxn_sb = qk.tile([128, 2 * S], FPR)

        # ---------------- phase 1: projections + RoPE ----------------
        with ExitStack() as pctx:
            xpool = pctx.enter_context(tc.tile_pool(name="xs", bufs=5))
            rpool = pctx.enter_context(tc.tile_pool(name="rope", bufs=2))
            pps = pctx.enter_context(tc.tile_pool(name="pps", bufs=8, space="PSUM"))

            for x_d, w_sb, dst_sb in ((xqT, wq_sb, qt_sb),
                                      (xkT, wk_sb, kt_sb)):
                psums = [pps.tile([128, 512], FP, tag="pj", name=f"pj{i}") for i in range(8)]
                for k in range(KT):
                    xk = xpool.tile([128, S], FPR, tag="x")
                    dma_eng = nc.sync if k % 2 == 0 else nc.scalar
                    for sc in range(4):
                        dma_eng.dma_start(xk[:, sc * 512:(sc + 1) * 512],
                                          x_d[k * 128:(k + 1) * 128,
                                              sc * 512:(sc + 1) * 512])
                    for t in range(2):
                        for sc in range(4):
                            _mm(nc, psums[t * 4 + sc][:],
                                w_sb[:, k * F + t * 128: k * F + (t + 1) * 128],
                                xk[:, sc * 512:(sc + 1) * 512],
                                start=(k == 0), stop=(k == KT - 1))
                for t in range(2):
                    for sc in range(4):
                        cp_eng = nc.vector if sc % 2 == 0 else nc.scalar
                        if cp_eng is nc.vector:
                            cp_eng.tensor_copy(
                                dst_sb[:, t * S + sc * 512: t * S + (sc + 1) * 512],
                                psums[t * 4 + sc][:])
                        else:
                            cp_eng.copy(
                                dst_sb[:, t * S + sc * 512: t * S + (sc + 1) * 512],
                                psums[t * 4 + sc][:])

                # RoPE (Q and K), per 128-row tile t
                if True:
                    for t in range(2):
                        lo, hi = t * S, (t + 1) * S
                        rot = rpool.tile([128, S], FPR, tag="rot")
                        # rotate-half across partitions: [0:32]<-[32:64],
                        # [32:64]<-[0:32], [64:96]<-[96:128], [96:128]<-[64:96]
                        for dst0, src0 in ((0, 32), (32, 0), (64, 96), (96, 64)):
                            nc.scalar.dma_start(rot[dst0:dst0 + 32, :],
                                                dst_sb[src0:src0 + 32, lo:hi])
                        nc.vector.tensor_mul(rot[:], rot[:], sin_sb[:])
                        nc.vector.tensor_mul(dst_sb[:, lo:hi], dst_sb[:, lo:hi], cos_sb[:])
                        nc.vector.tensor_add(dst_sb[:, lo:hi], dst_sb[:, lo:hi], rot[:])

            # V: out[s_tile, f] layout via column-slab x loads, st-outer,
            # one accumulation group per PSUM bank
            for st in range(ST):
                xslab = xpool.tile([128, KT * 128], FPR, tag="xv", name=f"xv{st}")
                dma_eng = nc.sync if st % 2 == 0 else nc.scalar
                dma_eng.dma_start(
                    xslab[:].rearrange("p (k c) -> p k c", k=KT),
                    xvT[:, st * 128:(st + 1) * 128].rearrange("(k p) c -> p k c", p=128))
                pv = pps.tile([128, 512], FP, tag="pj", name=f"pv{st}")
                for k in range(KT):
                    _mm(nc, pv[:, 0:256], xslab[:, k * 128:(k + 1) * 128],
                        wv_sb[:, k * F:(k + 1) * F],
                        start=(k == 0), stop=(k == KT - 1))
                c0 = st * HPG * 65
                dstv = v_sb[:, c0:c0 + HPG * 65].rearrange(
                    "p (h c) -> p h c", h=HPG)[:, :, 0:64]
                srcv = pv[:, 0:256].rearrange("p (h c) -> p h c", h=HPG)
                if st % 2 == 0:
                    nc.vector.tensor_copy(dstv, srcv)
                else:
                    nc.scalar.copy(dstv, srcv)

        # ---------------- phase 2: attention ----------------
        with ExitStack() as actx:
            sc_ps = actx.enter_context(tc.tile_pool(name="scps", bufs=3, space="PSUM"))
            ctx_ps = actx.enter_context(tc.tile_pool(name="ctxps", bufs=1, space="PSUM"))
            epool = actx.enter_context(tc.tile_pool(name="exp", bufs=6))
            npool = actx.enter_context(tc.tile_pool(name="norm", bufs=2))
            opool = actx.enter_context(tc.tile_pool(name="ost", bufs=4))
            if mask_mode == "general":
                bpool = actx.enter_context(tc.tile_pool(name="bias", bufs=2))

            ones_ap = v_sb[:].rearrange("p (b c) -> p b c", c=65)[:, :, 64:65]
            nc.vector.tensor_copy(ones_ap, ones64[:].rearrange("p (b o) -> p b o", o=1))

            for qc in range(NQC):
                for h in range(HPG):
                    t, po = h // 2, (h % 2) * 64
                    kt_hi = 8 * qc + 8 if mask_mode == "causal" else ST
                    last_b0 = min(kt_hi - 1, 8 * qc + 3) if mask_mode == "causal" else ST - 1
                    ctx_t = ctx_ps.tile([128, QCW], FP, tag="ctx")
                    qbase = t * S + qc * QCW

                    def emit_pv(e_t, kt, b0):
                        vcol = kt * HPG * 65 + h * 65
                        if b0:
                            _mm(nc, ctx_t[0:65, 0:512], v_sb[:, vcol:vcol + 65],
                                e_t[:, 0:512],
                                start=(kt == 0), stop=(kt == last_b0))
                        _mm(nc, ctx_t[0:65, 512:QCW], v_sb[:, vcol:vcol + 65],
                            e_t[:, 512:QCW],
                            start=(kt == 0), stop=(kt == kt_hi - 1))

                    # software pipeline: PV(kt-1) is emitted AFTER scores(kt)
                    # so the in-order PE stream never stalls waiting for
                    # exp(kt-1) with scores work available
                    pend = []
                    for kt in range(kt_hi):
                        j0 = max(0, kt * 128 - qc * QCW) if mask_mode == "causal" else 0
                        b0 = j0 < 512
                        rb0 = 0 if b0 else 512
                        kcol = t * S + kt * 128
                        s_ps = sc_ps.tile([128, QCW], FP, tag="sc")
                        if b0:
                            _mm(nc, s_ps[:, 0:512],
                                kt_sb[po:po + 64, kcol:kcol + 128],
                                qt_sb[po:po + 64, qbase:qbase + 512],
                                start=True, stop=True)
                        _mm(nc, s_ps[:, 512:QCW],
                            kt_sb[po:po + 64, kcol:kcol + 128],
                            qt_sb[po:po + 64, qbase + 512:qbase + QCW],
                            start=True, stop=True)
                        if mask_mode == "general":
                            bt = bpool.tile([128, QCW], FP, tag="bt")
                            nc.sync.dma_start(
                                bt[:], biasT[kt * 128:(kt + 1) * 128,
                                             qc * QCW:(qc + 1) * QCW])
                            nc.vector.tensor_add(s_ps[:], s_ps[:], bt[:])
                        if mask_mode == "causal" and kt * 128 >= qc * QCW:
                            if j0 > rb0:
                                nc.vector.memset(s_ps[:, rb0:j0], NEG)
                            nc.vector.tensor_add(s_ps[:, j0:j0 + 128],
                                                 s_ps[:, j0:j0 + 128], tri_sb[:])
                        e_t = epool.tile([128, QCW], FPR, tag="e")
                        nc.scalar.activation(e_t[:, rb0:QCW], s_ps[:, rb0:QCW], EXP)
                        pend.append((e_t, kt, b0))
                        if len(pend) > 3:
                            emit_pv(*pend.pop(0))
                    for p_ in pend:
                        emit_pv(*p_)
                    # normalize: rows 0:64 are ctx^T, row 64 is sum(exp)
                    r_sb = npool.tile([1, QCW], FPR, tag="r")
                    with nc.allow_low_precision(reason="float32r == fp32 width"):
                        nc.vector.reciprocal(r_sb[:], ctx_t[64:65, :])
                    # broadcast 1/sum across partitions via K=1 outer product
                    rb_ps = sc_ps.tile([64, QCW], FP, tag="sc", name="rbps")
                    for bank in range(2):
                        _mm(nc, rb_ps[:, bank * 512:(bank + 1) * 512], ones_col[:],
                            r_sb[:, bank * 512:(bank + 1) * 512], start=True, stop=True)
                    rb_sb = npool.tile([64, QCW], FP, tag="rb")
                    if h % 2 == 0:
                        nc.vector.tensor_copy(rb_sb[:], rb_ps[:])
                    else:
                        nc.scalar.copy(rb_sb[:], rb_ps[:])
                    nc.vector.tensor_mul(
                        ctxn_sb[po:po + 64, t * S + qc * QCW: t * S + (qc + 1) * QCW],
                        ctx_t[0:64, :], rb_sb[:])

                # output projection for this q-chunk
                for sti, st in enumerate(range(8 * qc, 8 * qc + 8)):
                    for ec in range(2):
                        o_ps = sc_ps.tile([128, 512], FP, tag="sc", name="ops")
                        for ft in range(2):
                            _mm(nc, o_ps[:],
                                ctxn_sb[:, ft * S + st * 128: ft * S + (st + 1) * 128],
                                wo_sb[:, ft * D + ec * 512: ft * D + (ec + 1) * 512],
                                start=(ft == 0), stop=(ft == 1))
                        o_sb = opool.tile([128, 512], FP, tag="o")
                        if (sti + ec) % 2 == 0:
                            nc.vector.tensor_copy(o_sb[:], o_ps[:])
                        else:
                            nc.scalar.copy(o_sb[:], o_ps[:])
                        out_eng = nc.sync if (sti + ec) % 2 == 0 else nc.scalar
                        out_eng.dma_start(
                            outp[st * 128:(st + 1) * 128, ec * 512:(ec + 1) * 512],
                            o_sb[:])
    _hoist_waits(nc)
    return nc


def _get_nc(mask_mode):
    if mask_mode not in _nc_cache:
        _nc_cache[mask_mode] = build_nc(mask_mode)
    return _nc_cache[mask_mode]


def _rope_tables():
    """cos/sin tables in [128, S] layout (64-row block tiled twice); sin is
    sign-folded for the rotate-half term."""
    inv_freq = (1.0 / (10000.0 ** (np.arange(0, DK, 2, dtype=np.float32) / np.float32(DK)))).astype(np.float32)
    t = np.arange(S, dtype=np.float32)
    freqs = np.outer(t, inv_freq).astype(np.float32)      # (S, 32)
    emb = np.concatenate([freqs, freqs], axis=-1)         # (S, 64)
    cos64 = np.cos(emb).T.astype(np.float32)              # (64, S)
    sin64 = np.sin(emb).T.astype(np.float32)
    sin64s = sin64.copy()
    sin64s[0:32] = -sin64[0:32]
    cos128 = np.ascontiguousarray(np.tile(cos64, (2, 1)))
    sin128 = np.ascontiguousarray(np.tile(sin64s, (2, 1)))
    return cos128, sin128


def _mask_mode(m2d):
    if (m2d != 0).all():
        return "full"
    if np.array_equal(m2d != 0, np.tril(np.ones((S, S), dtype=bool))):
        return "causal"
    return "general"


def _prepare(inputs):
    q = np.asarray(inputs["query"], dtype=np.float32)
    k = np.asarray(inputs["key"], dtype=np.float32)
    v = np.asarray(inputs["value"], dtype=np.float32)
    mask = np.asarray(inputs["mask"])
    Wq = np.asarray(inputs["W_q"], dtype=np.float32)
    Wk = np.asarray(inputs["W_k"], dtype=np.float32)
    Wv = np.asarray(inputs["W_v"], dtype=np.float32)
    Wo = np.asarray(inputs["W_o"], dtype=np.float32)

    modes = [_mask_mode(mask[b, 0]) for b in range(NB)]
    if all(m == "causal" for m in modes):
        mode = "causal"
    elif all(m == "full" for m in modes):
        mode = "full"
    else:
        mode = "general"
    nc = _get_nc(mode)

    cos128, sin128 = _rope_tables()
    scale = np.float32(1.0 / np.sqrt(DK))
    if mode == "causal":
        kk = np.arange(128)[:, None]
        qq = np.arange(128)[None, :]
        triD = np.where(kk <= qq, np.float32(0.0), np.float32(NEG)).astype(np.float32)

    xT = {}
    biasTs = {}
    for b in range(NB):
        xT[b] = (np.ascontiguousarray(q[b].T), np.ascontiguousarray(k[b].T),
                 np.ascontiguousarray(v[b].T))
        if mode == "general":
            biasTs[b] = np.where(mask[b, 0].T != 0, np.float32(0.0),
                                 np.float32(NEG)).astype(np.float32)

    in_maps = []
    for core in range(8):
        b, g = divmod(core, 4)
        rows = slice(g * F, (g + 1) * F)
        m = {
            "xqT": xT[b][0], "xkT": xT[b][1], "xvT": xT[b][2],
            "wqT": np.ascontiguousarray((Wq[rows] * scale).T),
            "wkT": np.ascontiguousarray(Wk[rows].T),
            "wvT": np.ascontiguousarray(Wv[rows].T),
            "woT": np.ascontiguousarray(Wo[:, rows].T),
            "cosS": cos128, "sinS": sin128,
        }
        if mode == "general":
            m["biasT"] = biasTs[b]
        if mode == "causal":
            m["triD"] = triD
        in_maps.append(m)
    return nc, in_maps


def _gather(res):
    out = np.zeros((NB, S, D), dtype=np.float32)
    for core in range(8):
        out[core // 4] += res.results[core]["outp"]
    return out


def kernel(**inputs):
    from concourse import bass_utils

    nc, in_maps = _prepare(inputs)
    res = bass_utils.run_bass_kernel_spmd(nc, in_maps, core_ids=list(range(8)))
    return _gather(res)


def run_traced(**inputs):
    """Run once with NTFF tracing; returns (out, exec_time_ns, raw results)."""
    from concourse import bass_utils

    nc, in_maps = _prepare(inputs)
    res = bass_utils.run_bass_kernel_spmd(nc, in_maps, core_ids=list(range(8)),
                                          trace=True)
    return _gather(res), res.exec_time_ns, res

